# revision 13
# baseline (speedup 1.0000x reference)
"""Trainium2 Bass kernel for nn_Block_54382875902076 (dense transformer block).

Reference computation (B=4, S=2048, E=512, H=8, D=64, fp32):
    res = x
    h   = LN1(x)                      (no bias, eps=1e-6)
    h   = res + Attn(h)               (causal, wo1 [H,D,E] then wo2 [E,E])
    h   = LN2(h)
    out = res + gelu(h @ w1) @ w2     (NOTE: res = block input, both residuals)

Sharding (8 cores): core c = (batch b = c//2, head-group g = c%2).
Each core computes LN1 + QKV for its 4 heads over the full sequence,
exact-causal attention (identical static structure on all cores — SPMD
requires one graph), then an E-partition-major back half: wo1 produces
o1^T directly from attnT (no transposes), pair-wise ReduceScatters over
a [2, E, tokens/2] layout hand each core its half of every block's
token columns, and wo2 + LN2 + MLP + the output all stay E-major (the
host reassembles [tokens, E] rows from the [128, 4, SQ] result).

Schedule: the attention phase is exp-bound on the Scalar engine
(~1.34 us per 128x1024 chunk vs ~0.65 us of PE work), so wo1/wo2/LN2
for earlier blocks are drip-fed one small PE op per attention chunk
into that slack instead of burst-issued (bursts stall the exp stream).
LN2 runs E-major via ones-vector colsum matmuls + row math + gpsimd
partition-broadcast. m1 accumulates into [128,1024] PSUM (2 banks) with
one batched gelu per mi; m2 is weight-stationary with both token halves
per LDWEIGHTS so weight loads hide under the matmul stream. A dummy
warm-up matmul burst at t=0 trips the PE HAM clock gate to 2.4 GHz
while the first DMAs land.

Key measured-on-this-fleet choices kept from the earlier version:
 - all matmuls bf16 (fp32 is 4x slower on the PE); f32 stats/accumulation
 - attention computed scores-transposed [k, q]: no P-matrix transpose, the
   softmax denominator rides as a 65th ones-row in V, exact causal extents
   (extent(qt) = 4(qt+1) chunks) with 4 static diagonal mask tiles,
   score matmuls row-packed two heads at a time (64-row tile_position)
 - AV is software-pipelined one chunk behind the scores
 - rsqrt = exp(-0.5*ln(v+eps)) and a filtered activation-table list keep
   LayerNorm+softmax in ONE ACT table set
 - softmax 1/denominator: DVE reciprocal is ~6.5 cyc/elem/lane, so the
   [1,512] row is DMA-scattered over 8 partitions first
 - host pre-arranges weights/x partition-major so DMAs are few big-packet
   transfers; x is loaded once as bf16 [128, 16, 512]
"""

import functools
import sys

import numpy as np

for _p in ("/opt/trn_rl_repo", "/root/.axon_site/_ro/trn_rl_repo"):
    if _p not in sys.path:
        sys.path.append(_p)

import ml_dtypes  # noqa: E402
import concourse.bass as bass  # noqa: E402
import concourse.tile as tile  # noqa: E402
from concourse import bacc, mybir  # noqa: E402
from concourse.bass_utils import run_bass_kernel_spmd  # noqa: E402

_ALLOWED_ACT_SETS = {"natural_log_exp_and_others", "gelu_apprx_tanh_and_others"}
_orig_get_act_tables = bacc.get_activation_tables


def _filtered_act_tables(module_arch):
    tabs = _orig_get_act_tables(module_arch)
    return {
        name: (funcs if name in _ALLOWED_ACT_SETS else set())
        for name, funcs in tabs.items()
    }


bacc.get_activation_tables = _filtered_act_tables

F32 = mybir.dt.float32
BF16 = mybir.dt.bfloat16
AF = mybir.ActivationFunctionType
ALU = mybir.AluOpType

B, S, E, H, D = 4, 2048, 512, 8, 64
HG = H // 2            # heads per core
SQ = S // 2            # tokens per core after reduce-scatter
NT = S // 128          # 16 token tiles (full seq)
QTS = S // 512         # 4 q-tiles of 512 for attention


def _build_graph():
    nc = bacc.Bacc("TRN2", target_bir_lowering=False, debug=False, num_devices=8)

    xf = nc.declare_dram_parameter("xf", [128, NT, E], BF16, isOutput=False)
    xqT = nc.declare_dram_parameter("xqT", [128, 4, SQ], F32, isOutput=False)
    wq = nc.declare_dram_parameter("wq", [128, 4, HG * D], BF16, isOutput=False)
    wk = nc.declare_dram_parameter("wk", [128, 4, HG * D], BF16, isOutput=False)
    wv = nc.declare_dram_parameter("wv", [128, 4, HG * D], BF16, isOutput=False)
    wo1 = nc.declare_dram_parameter("wo1", [128, 2, E], BF16, isOutput=False)
    wo2 = nc.declare_dram_parameter("wo2", [128, 4, E], BF16, isOutput=False)
    w1 = nc.declare_dram_parameter("w1", [128, 4, 4 * E], BF16, isOutput=False)
    w2 = nc.declare_dram_parameter("w2", [128, 16, E], BF16, isOutput=False)
    masks = nc.declare_dram_parameter("masks", [128, 4, 512], BF16, isOutput=False)
    out = nc.declare_dram_parameter("out", [128, 4, SQ], F32, isOutput=True)

    with tile.TileContext(nc) as tc:
        with (
            tc.tile_pool(name="consts", bufs=1) as consts,
            tc.tile_pool(name="acts", bufs=1) as acts,
            tc.tile_pool(name="xring", bufs=2) as xring,
            tc.tile_pool(name="work", bufs=2) as work,
            tc.tile_pool(name="stats", bufs=6) as stats,
            tc.tile_pool(name="den", bufs=2) as den,
            tc.tile_pool(name="lnw", bufs=5) as lnw,
            tc.tile_pool(name="expp", bufs=3) as expp,
            tc.tile_pool(name="outb", bufs=2) as outp,
            tc.tile_pool(name="psA", bufs=2, space="PSUM") as psA,
            tc.tile_pool(name="psB", bufs=2, space="PSUM") as psB,
            tc.tile_pool(name="psC", bufs=2, space="PSUM") as psC,
            tc.tile_pool(name="dram", bufs=1, space="DRAM") as dram,
        ):
            # ---- constants / identity / warm-up --------------------------
            eps_t = consts.tile([128, 1], F32)
            nc.vector.memset(eps_t, 1e-6)
            ones512 = consts.tile([128, 1], BF16)
            nc.vector.memset(ones512, 1.0 / 512.0)
            ident = consts.tile([128, 128], BF16)
            from concourse.masks import make_identity
            make_identity(nc, ident[:])

            # dummy matmul burst: ~3.5us of PE activity trips the HAM clock
            # gate to 2.4 GHz while the first weight/x DMAs are in flight
            warm = psC.tile([128, 128], F32, tag="psC", name="warmup")
            for wi in range(28):
                nc.tensor.matmul(
                    warm[:], lhsT=ident[:], rhs=ident[:],
                    start=(wi == 0), stop=(wi == 27),
                )

            def load_const(shape, src, tag):
                t = consts.tile(shape, BF16, tag=tag)
                nc.gpsimd.dma_start(t[:], src[:])
                return t

            # attention-critical weights first, MLP weights + residual last
            wq_sb = load_const([128, 4, HG * D], wq, "wq_sb")
            wk_sb = load_const([128, 4, HG * D], wk, "wk_sb")
            wv_sb = load_const([128, 4, HG * D], wv, "wv_sb")
            masks_sb = load_const([128, 4, 512], masks, "masks_sb")
            wo1_sb = load_const([128, 2, E], wo1, "wo1_sb")
            wo2_sb = load_const([128, 4, E], wo2, "wo2_sb")
            w1_sb = load_const([128, 4, 4 * E], w1, "w1_sb")
            w2_sb = load_const([128, 16, E], w2, "w2_sb")
            xqT_sb = acts.tile([128, 4, SQ], F32)
            nc.gpsimd.dma_start(xqT_sb[:], xqT[:])

            def layernorm_tile(src_ap, dst_tile):
                """dst (bf16) = (src - mean) * rsqrt(var + eps); stats in fp32."""
                st6 = stats.tile([128, 6], F32, tag="st6")
                nc.vector.bn_stats(st6[:], src_ap)
                mv = stats.tile([128, 2], F32, tag="mv")
                nc.vector.bn_aggr(mv[:], st6[:])
                lnv = stats.tile([128, 1], F32, tag="lnv")
                nc.scalar.activation(lnv[:], mv[:, 1:2], AF.Ln, bias=eps_t[:])
                rsig = stats.tile([128, 1], F32, tag="rsig")
                nc.scalar.activation(rsig[:], lnv[:], AF.Exp, scale=-0.5)
                nc.vector.tensor_scalar(
                    dst_tile[:], src_ap, mv[:, 0:1], rsig[:],
                    op0=ALU.subtract, op1=ALU.mult,
                )

            # ---- LN1 + per-block transpose + QKV -------------------------
            # x arrives through a 2-deep ring of 512-token chunks (prefetch
            # one block ahead) instead of a persistent [128,16,512] tile.
            h1T = acts.tile([128, 4, S], BF16)
            KT = acts.tile([128, 2, S], BF16)
            QT = acts.tile([128, 2, S], BF16)
            V65 = acts.tile([128, NT, HG, D + 1], BF16)
            nc.vector.memset(V65[:, :, :, D:D + 1], 1.0)

            def xf_fetch(st):
                xch = xring.tile([128, 4, E], BF16, tag="xch", name=f"xch{st}")
                nc.sync.dma_start(xch[:], xf[:, 4 * st:4 * st + 4, :])
                return xch

            xch_cur = xf_fetch(0)
            for st in range(4):
                xch = xch_cur
                if st < 3:
                    xch_cur = xf_fetch(st + 1)
                h1ts0 = []
                for lt4 in range(4):
                    t = 4 * st + lt4
                    h1t = lnw.tile([128, E], BF16, tag="lnt", name=f"h1t{t}")
                    layernorm_tile(xch[:, lt4, :], h1t)
                    h1ts0.append(h1t)
                for lt in range(4):
                    for ko in range(4):
                        psT = psC.tile([128, 128], BF16, tag="psC",
                                       name=f"psH{st}_{lt}_{ko}")
                        nc.tensor.transpose(
                            psT[:], h1ts0[lt][:, ko * 128:(ko + 1) * 128], ident[:]
                        )
                        nc.vector.tensor_copy(
                            h1T[:, ko, st * 512 + lt * 128:st * 512 + (lt + 1) * 128],
                            psT[:],
                        )
                sl = slice(st * 512, (st + 1) * 512)
                for mi in range(2):
                    for dst, w_sb in ((KT, wk_sb), (QT, wq_sb)):
                        ps = psC.tile([128, 512], F32, tag="psC")
                        for ko in range(4):
                            nc.tensor.matmul(
                                ps[:],
                                lhsT=w_sb[:, ko, mi * 128:(mi + 1) * 128],
                                rhs=h1T[:, ko, sl],
                                start=(ko == 0), stop=(ko == 3),
                            )
                        nc.vector.tensor_copy(dst[:, mi, sl], ps[:])
                for tt in range(4 * st, 4 * st + 4):
                    ps = psC.tile([128, 512], F32, tag="psC")
                    for ko in range(4):
                        nc.tensor.matmul(
                            ps[:, 0:HG * D],
                            lhsT=h1T[:, ko, tt * 128:(tt + 1) * 128],
                            rhs=wv_sb[:, ko, :],
                            start=(ko == 0), stop=(ko == 3),
                        )
                    nc.vector.tensor_copy(
                        V65[:, tt, :, 0:D],
                        ps[:, 0:HG * D].rearrange("p (h d) -> p h d", h=HG),
                    )

            # ---- attention state + E-major back-half tiles ---------------
            attnT = acts.tile([128, 2, S], BF16)
            # RS layout: row ((qt*2 + g)*E + e), 256 token cols — axis-0 split
            # of each block's [2*E, 256] region hands pair-rank g its tokens
            o1T_dram = dram.tile([QTS * 2 * E, 256], BF16)
            o1rT_dram = dram.tile([QTS * E, 256], BF16)
            o1rT = acts.tile([128, 4, SQ], BF16)
            h2Tb = acts.tile([128, 4, SQ], BF16)
            h2ln = acts.tile([128, 4, SQ], BF16)
            m1T = acts.tile([128, 16, 512], BF16)   # one token-half at a time

            def attention_block(qt, bg):
                """bg: list of closures; one is issued per (a, chunk) slot to
                fill the PE slack under the exp stream."""
                ext = 4 * (qt + 1)           # causal extent in 128-chunks
                for a in range(2):           # local head pairs (2a, 2a+1)
                    avA = psB.tile([D + 1, 512], F32, tag="psB")
                    avB = psB.tile([D + 1, 512], F32, tag="psB")
                    for c in range(ext):
                        sp = psA.tile([128, 1024], F32, tag="psA")
                        nc.tensor.matmul(
                            sp[:, 0:512],
                            lhsT=KT[0:64, a, c * 128:(c + 1) * 128],
                            rhs=QT[0:64, a, qt * 512:(qt + 1) * 512],
                            start=True, stop=True,
                        )
                        nc.tensor.matmul(
                            sp[:, 512:1024],
                            lhsT=KT[64:128, a, c * 128:(c + 1) * 128],
                            rhs=QT[64:128, a, qt * 512:(qt + 1) * 512],
                            start=True, stop=True,
                        )
                        ex = expp.tile([128, 1024], BF16, tag="ex")
                        nc.scalar.activation(ex[:], sp[:], AF.Exp, scale=D ** -0.5)
                        j = c - 4 * qt
                        if j >= 0:           # diagonal chunk: apply causal mask
                            nc.vector.tensor_mul(ex[:, 0:512], ex[:, 0:512], masks_sb[:, j, :])
                            nc.vector.tensor_mul(ex[:, 512:1024], ex[:, 512:1024], masks_sb[:, j, :])
                        nc.tensor.matmul(
                            avA[:], lhsT=V65[:, c, 2 * a, :], rhs=ex[:, 0:512],
                            start=(c == 0), stop=(c == ext - 1),
                        )
                        nc.tensor.matmul(
                            avB[:], lhsT=V65[:, c, 2 * a + 1, :], rhs=ex[:, 512:1024],
                            start=(c == 0), stop=(c == ext - 1),
                        )
                        if bg:
                            bg.pop(0)()
                    for hh, av in ((2 * a, avA), (2 * a + 1, avB)):
                        # copy PSUM out quickly, then build 1/denominator with
                        # the free dim spread across partitions (reciprocal is
                        # ~6.5 cyc per free-elem per lane, so [1,512] is slow)
                        avs = work.tile([D + 1, 512], F32, tag="avs")
                        nc.vector.tensor_copy(avs[:], av[:])
                        d4 = den.tile([8, 64], F32, tag="d4")
                        nc.sync.dma_start(
                            d4[:], avs[D:D + 1, :].rearrange("o (p f) -> o p f", p=8)
                        )
                        r4 = den.tile([8, 64], F32, tag="r4")
                        nc.vector.reciprocal(r4[:], d4[:])
                        rrow = den.tile([1, 512], F32, tag="rrow")
                        nc.sync.dma_start(
                            rrow.rearrange("o (p f) -> o p f", p=8), r4[:]
                        )
                        den_b = work.tile([64, 512], F32, tag="denb")
                        nc.gpsimd.partition_broadcast(den_b[:], rrow[0:1, :], channels=64)
                        if hh % 2 == 0:
                            nc.vector.tensor_tensor(
                                attnT[0:64, a, qt * 512:(qt + 1) * 512],
                                avs[0:D, :], den_b[:], op=ALU.mult,
                            )
                        else:
                            tmp = work.tile([64, 512], BF16, tag="atmp")
                            nc.vector.tensor_tensor(tmp[:], avs[0:D, :], den_b[:], op=ALU.mult)
                            nc.sync.dma_start(attnT[64:128, a, qt * 512:(qt + 1) * 512], tmp[:])

            # ---- E-major wo1 + per-block ReduceScatter -------------------
            def wo1_eo_group(qt, eo):
                """o1T[eo-chunk, block qt] = sum_a wo1_a^T @ attnT_a, then
                cast + store to the token-split RS layout."""
                ps = psC.tile([128, 512], F32, tag="psC")
                for a in range(2):
                    nc.tensor.matmul(
                        ps[:],
                        lhsT=wo1_sb[:, a, eo * 128:(eo + 1) * 128],
                        rhs=attnT[:, a, qt * 512:(qt + 1) * 512],
                        start=(a == 0), stop=(a == 1),
                    )
                o1t = work.tile([128, 512], BF16, tag="wbf")
                nc.vector.tensor_copy(o1t[:], ps[:])
                for g in range(2):
                    base = (qt * 2 + g) * E + eo * 128
                    nc.gpsimd.dma_start(
                        o1T_dram[base:base + 128, :],
                        o1t[:, g * 256:(g + 1) * 256],
                    )

            def rs_block(qt):
                nc.gpsimd.collective_compute(
                    "ReduceScatter", ALU.add,
                    replica_groups=[[0, 1], [2, 3], [4, 5], [6, 7]],
                    ins=[o1T_dram[qt * 2 * E:(qt + 1) * 2 * E, :].opt()],
                    outs=[o1rT_dram[qt * E:(qt + 1) * E, :].opt()],
                )

            def o1rT_load(qt):
                nc.gpsimd.dma_start(
                    o1rT[:, :, qt * 256:(qt + 1) * 256],
                    o1rT_dram[qt * E:(qt + 1) * E, :].rearrange(
                        "(eo p) t -> p eo t", p=128
                    ),
                )

            # ---- E-major wo2 + LN2 (token half h = 512 cols) -------------
            def wo2_half_eo(h, eo, part):
                """half of one eo-chunk of wo2 (2 MMs) — drip-sized."""
                if part == 0:
                    ps = psC.tile([128, 512], F32, tag="psC", name=f"wo2_{h}_{eo}")
                    wo2_half_eo.ps[(h, eo)] = ps
                else:
                    ps = wo2_half_eo.ps.pop((h, eo))
                for ei in (0, 1) if part == 0 else (2, 3):
                    nc.tensor.matmul(
                        ps[:],
                        lhsT=wo2_sb[:, ei, eo * 128:(eo + 1) * 128],
                        rhs=o1rT[:, ei, h * 512:(h + 1) * 512],
                        start=(ei == 0), stop=(ei == 3),
                    )
                if part == 1:
                    nc.vector.tensor_tensor(
                        h2Tb[:, eo, h * 512:(h + 1) * 512],
                        ps[:], xqT_sb[:, eo, h * 512:(h + 1) * 512], op=ALU.add,
                    )
            wo2_half_eo.ps = {}

            def ln2_stats_mm(h, which):
                """colsum matmuls: which=0 -> mean, which=1 -> E[x^2]."""
                ps = psC.tile([1, 512], F32, tag="psC", name=f"ln2s_{h}_{which}")
                if which == 0:
                    for ei in range(4):
                        nc.tensor.matmul(
                            ps[:], lhsT=ones512[:],
                            rhs=h2Tb[:, ei, h * 512:(h + 1) * 512],
                            start=(ei == 0), stop=(ei == 3),
                        )
                    ln2_stats_mm.mu[h] = ps
                else:
                    for ei in range(4):
                        sq = work.tile([128, 512], BF16, tag="sqw")
                        nc.vector.tensor_tensor(
                            sq[:], h2Tb[:, ei, h * 512:(h + 1) * 512],
                            h2Tb[:, ei, h * 512:(h + 1) * 512], op=ALU.mult,
                        )
                        nc.tensor.matmul(
                            ps[:], lhsT=ones512[:], rhs=sq[:],
                            start=(ei == 0), stop=(ei == 3),
                        )
                    ln2_stats_mm.ex2[h] = ps
            ln2_stats_mm.mu = {}
            ln2_stats_mm.ex2 = {}

            def ln2_rowmath(h):
                # sequential [1,512] scratch shares one ring tag (each alloc
                # only reads the previous one, so a 2-deep ring is safe)
                ps_mu = ln2_stats_mm.mu.pop(h)
                ps_ex2 = ln2_stats_mm.ex2.pop(h)
                mu_sb = den.tile([1, 512], F32, tag="musb", name=f"musb{h}")
                nc.vector.tensor_copy(mu_sb[:], ps_mu[:])
                # two PSUM sources on one DVE op are illegal, hence mu_sb
                musq = den.tile([1, 512], F32, tag="row", name=f"musq{h}")
                nc.vector.tensor_tensor(musq[:], mu_sb[:], mu_sb[:], op=ALU.mult)
                var_s = den.tile([1, 512], F32, tag="row", name=f"vars{h}")
                nc.vector.tensor_tensor(var_s[:], ps_ex2[:], musq[:], op=ALU.subtract)
                lnv = den.tile([1, 512], F32, tag="row", name=f"lnv2{h}")
                nc.scalar.activation(lnv[:], var_s[:], AF.Ln, bias=eps_t[0:1])
                r_f = den.tile([1, 512], F32, tag="row", name=f"rf{h}")
                nc.scalar.activation(r_f[:], lnv[:], AF.Exp, scale=-0.5)
                mur = den.tile([1, 512], BF16, tag="mur")
                nc.vector.tensor_tensor(mur[:], mu_sb[:], r_f[:], op=ALU.mult)
                r_bf = den.tile([1, 512], BF16, tag="rbf")
                nc.vector.tensor_copy(r_bf[:], r_f[:])
                r_b = den.tile([128, 512], BF16, tag="rb", name=f"rb{h}")
                nc.gpsimd.partition_broadcast(r_b[:], r_bf[0:1, :], channels=128)
                mur_b = den.tile([128, 512], BF16, tag="murb", name=f"murb{h}")
                nc.gpsimd.partition_broadcast(mur_b[:], mur[0:1, :], channels=128)
                ln2_rowmath.bcast[h] = (r_b, mur_b)
            ln2_rowmath.bcast = {}

            def ln2_apply_eo(h, eo):
                r_b, mur_b = ln2_rowmath.bcast[h]
                tmp = work.tile([128, 512], BF16, tag="sqw")
                nc.vector.tensor_tensor(
                    tmp[:], h2Tb[:, eo, h * 512:(h + 1) * 512], r_b[:], op=ALU.mult
                )
                nc.vector.tensor_tensor(
                    h2ln[:, eo, h * 512:(h + 1) * 512], tmp[:], mur_b[:],
                    op=ALU.subtract,
                )

            # ---- MLP (E-major, weight-stationary) ------------------------
            def m1_half(h):
                for mi in range(16):
                    ps = psA.tile([128, 512], F32, tag="psA", name=f"m1_{h}_{mi}")
                    for ko in range(4):
                        nc.tensor.matmul(
                            ps[:],
                            lhsT=w1_sb[:, ko, mi * 128:(mi + 1) * 128],
                            rhs=h2ln[:, ko, h * 512:(h + 1) * 512],
                            start=(ko == 0), stop=(ko == 3),
                        )
                    nc.scalar.activation(m1T[:, mi, :], ps[:], AF.Gelu_apprx_tanh)

            def m2_half(h):
                for eo in range(4):
                    ps = psC.tile([128, 512], F32, tag="psC", name=f"m2_{h}_{eo}")
                    for ko in range(16):
                        nc.tensor.matmul(
                            ps[:],
                            lhsT=w2_sb[:, ko, eo * 128:(eo + 1) * 128],
                            rhs=m1T[:, ko, :],
                            start=(ko == 0), stop=(ko == 15),
                        )
                    ob = outp.tile([128, 512], F32, tag="outb")
                    nc.vector.tensor_tensor(
                        ob[:], ps[:], xqT_sb[:, eo, h * 512:(h + 1) * 512], op=ALU.add
                    )
                    nc.gpsimd.dma_start(out[:, eo, h * 512:(h + 1) * 512], ob[:])

            # ---- schedule ------------------------------------------------
            # drips are padded with no-op slots so a drip's producer (the
            # previous block's attnT / an RS) is finished before the PE — an
            # in-order engine — reaches the drip's matmuls and would stall.
            nop = lambda: None
            attention_block(0, [])
            attention_block(1, [nop] * 8
                            + [lambda eo=eo: wo1_eo_group(0, eo) for eo in range(4)]
                            + [lambda: rs_block(0)])
            attention_block(2, [nop] * 8
                            + [lambda eo=eo: wo1_eo_group(1, eo) for eo in range(4)]
                            + [lambda: rs_block(1)]
                            + [lambda: o1rT_load(0), lambda: o1rT_load(1)])
            bg3 = [nop] * 4
            bg3 += [lambda eo=eo: wo1_eo_group(2, eo) for eo in range(4)]
            bg3 += [lambda: rs_block(2)]
            for eo in range(4):
                bg3 += [lambda eo=eo: wo2_half_eo(0, eo, 0),
                        lambda eo=eo: wo2_half_eo(0, eo, 1)]
            bg3 += [lambda: ln2_stats_mm(0, 0), lambda: ln2_stats_mm(0, 1)]
            bg3 += [lambda: ln2_rowmath(0)]
            bg3 += [lambda eo=eo: ln2_apply_eo(0, eo) for eo in range(4)]
            attention_block(3, bg3)

            # ---- tail ----------------------------------------------------
            for eo in range(4):
                wo1_eo_group(3, eo)
            rs_block(3)
            o1rT_load(2)
            o1rT_load(3)
            m1_half(0)                      # runs on PE while RS(3) is in flight
            for eo in range(4):
                wo2_half_eo(1, eo, 0)
                wo2_half_eo(1, eo, 1)
            ln2_stats_mm(1, 0)
            ln2_stats_mm(1, 1)
            ln2_rowmath(1)
            for eo in range(4):
                ln2_apply_eo(1, eo)         # DVE — issued before m2(0)'s adds
            m2_half(0)                      # so it overlaps m2(0)'s matmuls
            m1_half(1)
            m2_half(1)

    nc.finalize()
    return nc


@functools.lru_cache(maxsize=1)
def _get_graph():
    return _build_graph()


def _bf16_kpm(a, p=128):
    """[K, M] fp32 -> contiguous [p, K//p, M] bf16 (SBUF (k p) layout)."""
    k, m = a.shape
    return np.ascontiguousarray(
        a.reshape(k // p, p, m).transpose(1, 0, 2)
    ).astype(ml_dtypes.bfloat16)


def _own_rows(rank):
    """Global row indices owned by a core after the per-block reduce-scatters."""
    return np.concatenate(
        [np.arange(512 * qt + 256 * rank, 512 * qt + 256 * rank + 256) for qt in range(QTS)]
    )


def _make_in_maps(x, wq, wk, wv, wo1, wo2, w1, w2, ln1_scale, ln2_scale):
    x = np.asarray(x, dtype=np.float32)
    wq = np.asarray(wq, dtype=np.float32).reshape(E, H * D)
    wk = np.asarray(wk, dtype=np.float32).reshape(E, H * D)
    wv = np.asarray(wv, dtype=np.float32).reshape(E, H * D)
    wo1 = np.asarray(wo1, dtype=np.float32).reshape(H * D, E)
    wo2 = np.asarray(wo2, dtype=np.float32)
    w1 = np.asarray(w1, dtype=np.float32)
    w2 = np.asarray(w2, dtype=np.float32)
    s1 = np.asarray(ln1_scale, dtype=np.float32)[:, None]
    s2 = np.asarray(ln2_scale, dtype=np.float32)[:, None]

    wq_s, wk_s, wv_s = s1 * wq, s1 * wk, s1 * wv
    w1_s = s2 * w1

    # causal mask patterns for diagonal 128-chunks within a 512 q-tile:
    # mask_j[p, f] = 1.0 iff (128j + p) <= f;  stored [p, j, f]
    iota_p = np.arange(128)[:, None]
    iota_f = np.arange(512)[None, :]
    mask_np = np.ascontiguousarray(np.stack(
        [(128 * j + iota_p <= iota_f).astype(np.float32) for j in range(4)]
    ).transpose(1, 0, 2)).astype(ml_dtypes.bfloat16)

    in_maps = []
    for c in range(8):
        b, g = c // 2, c % 2
        hd = slice(g * HG * D, (g + 1) * HG * D)
        rows = _own_rows(c % 2)
        # E-major residual for the owned rows: [128, 4, SQ] with e = eo*128+p
        xqT_arr = np.ascontiguousarray(
            x[b][rows].T.reshape(4, 128, SQ).transpose(1, 0, 2)
        )
        in_maps.append({
            "xf": np.ascontiguousarray(x[b].reshape(NT, 128, E).transpose(1, 0, 2)).astype(ml_dtypes.bfloat16),
            "xqT": xqT_arr,
            "wq": _bf16_kpm(wq_s[:, hd]),
            "wk": _bf16_kpm(wk_s[:, hd]),
            "wv": _bf16_kpm(wv_s[:, hd]),
            "wo1": _bf16_kpm(wo1[hd, :]),
            "wo2": _bf16_kpm(wo2),
            "w1": _bf16_kpm(w1_s),
            "w2": _bf16_kpm(w2),
            "masks": mask_np,
        })
    return in_maps


def run(trace=False, **inputs):
    nc = _get_graph()
    in_maps = _make_in_maps(**inputs)
    res = run_bass_kernel_spmd(nc, in_maps, core_ids=list(range(8)), trace=trace)
    y = np.empty((B, S, E), dtype=np.float32)
    for c in range(8):
        b = c // 2
        # out is E-major [128 p, 4 eo, SQ t]; rows want [t, eo*128+p]
        y[b][_own_rows(c % 2)] = res.results[c]["out"].transpose(2, 1, 0).reshape(SQ, E)
    return y, res


def kernel(**inputs):
    y, _ = run(trace=False, **inputs)
    return y


# revision 14
# speedup vs baseline: 1.0690x; 1.0690x over previous
"""Trainium2 Bass kernel for nn_Block_54382875902076 (dense transformer block).

Reference computation (B=4, S=2048, E=512, H=8, D=64, fp32):
    res = x
    h   = LN1(x)                      (no bias, eps=1e-6)
    h   = res + Attn(h)               (causal, wo1 [H,D,E] then wo2 [E,E])
    h   = LN2(h)
    out = res + gelu(h @ w1) @ w2     (NOTE: res = block input, both residuals)

Sharding (8 cores): core c = (batch b = c//2, head-group g = c%2).
Each core computes LN1 + QKV for its 4 heads over the full sequence,
exact-causal attention (identical static structure on all cores — SPMD
requires one graph), then an E-partition-major back half: wo1 produces
o1^T directly from attnT (no transposes), pair-wise ReduceScatters over
a [2, E, tokens/2] layout hand each core its half of every block's
token columns, and wo2 + LN2 + MLP + the output all stay E-major (the
host reassembles [tokens, E] rows from the [128, 4, SQ] result).

Schedule: the attention phase is exp-bound on the Scalar engine
(~1.34 us per 128x1024 chunk vs ~0.65 us of PE work), so wo1/wo2/LN2
for earlier blocks are drip-fed one small PE op per attention chunk
into that slack instead of burst-issued (bursts stall the exp stream).
LN2 runs E-major via ones-vector colsum matmuls + row math + gpsimd
partition-broadcast. m1 accumulates into [128,1024] PSUM (2 banks) with
one batched gelu per mi; m2 is weight-stationary with both token halves
per LDWEIGHTS so weight loads hide under the matmul stream. A dummy
warm-up matmul burst at t=0 trips the PE HAM clock gate to 2.4 GHz
while the first DMAs land.

Key measured-on-this-fleet choices kept from the earlier version:
 - all matmuls bf16 (fp32 is 4x slower on the PE); f32 stats/accumulation
 - attention computed scores-transposed [k, q]: no P-matrix transpose, the
   softmax denominator rides as a 65th ones-row in V, exact causal extents
   (extent(qt) = 4(qt+1) chunks) with 4 static diagonal mask tiles,
   score matmuls row-packed two heads at a time (64-row tile_position)
 - AV is software-pipelined one chunk behind the scores
 - rsqrt = exp(-0.5*ln(v+eps)) and a filtered activation-table list keep
   LayerNorm+softmax in ONE ACT table set
 - softmax 1/denominator: DVE reciprocal is ~6.5 cyc/elem/lane, so the
   [1,512] row is DMA-scattered over 8 partitions first
 - host pre-arranges weights/x partition-major so DMAs are few big-packet
   transfers; x is loaded once as bf16 [128, 16, 512]
"""

import functools
import sys

import numpy as np

for _p in ("/opt/trn_rl_repo", "/root/.axon_site/_ro/trn_rl_repo"):
    if _p not in sys.path:
        sys.path.append(_p)

import ml_dtypes  # noqa: E402
import concourse.bass as bass  # noqa: E402
import concourse.tile as tile  # noqa: E402
from concourse import bacc, mybir  # noqa: E402
from concourse.bass_utils import run_bass_kernel_spmd  # noqa: E402

_ALLOWED_ACT_SETS = {"natural_log_exp_and_others", "gelu_apprx_tanh_and_others"}
_orig_get_act_tables = bacc.get_activation_tables


def _filtered_act_tables(module_arch):
    tabs = _orig_get_act_tables(module_arch)
    return {
        name: (funcs if name in _ALLOWED_ACT_SETS else set())
        for name, funcs in tabs.items()
    }


bacc.get_activation_tables = _filtered_act_tables

F32 = mybir.dt.float32
BF16 = mybir.dt.bfloat16
AF = mybir.ActivationFunctionType
ALU = mybir.AluOpType

B, S, E, H, D = 4, 2048, 512, 8, 64
HG = H // 2            # heads per core
SQ = S // 2            # tokens per core after reduce-scatter
NT = S // 128          # 16 token tiles (full seq)
QTS = S // 512         # 4 q-tiles of 512 for attention


def _build_graph():
    nc = bacc.Bacc("TRN2", target_bir_lowering=False, debug=False, num_devices=8)

    xf = nc.declare_dram_parameter("xf", [128, NT, E], BF16, isOutput=False)
    xqT = nc.declare_dram_parameter("xqT", [128, 4, SQ], F32, isOutput=False)
    wq = nc.declare_dram_parameter("wq", [128, 4, HG * D], BF16, isOutput=False)
    wk = nc.declare_dram_parameter("wk", [128, 4, HG * D], BF16, isOutput=False)
    wv = nc.declare_dram_parameter("wv", [128, 4, HG * D], BF16, isOutput=False)
    wo1 = nc.declare_dram_parameter("wo1", [128, 2, E], BF16, isOutput=False)
    wo2 = nc.declare_dram_parameter("wo2", [128, 4, E], BF16, isOutput=False)
    w1 = nc.declare_dram_parameter("w1", [128, 4, 4 * E], BF16, isOutput=False)
    w2 = nc.declare_dram_parameter("w2", [128, 16, E], BF16, isOutput=False)
    masks = nc.declare_dram_parameter("masks", [128, 4, 512], BF16, isOutput=False)
    out = nc.declare_dram_parameter("out", [128, 4, SQ], F32, isOutput=True)

    with tile.TileContext(nc) as tc:
        with (
            tc.tile_pool(name="consts", bufs=1) as consts,
            tc.tile_pool(name="acts", bufs=1) as acts,
            tc.tile_pool(name="xring", bufs=2) as xring,
            tc.tile_pool(name="work", bufs=2) as work,
            tc.tile_pool(name="stats", bufs=6) as stats,
            tc.tile_pool(name="den", bufs=2) as den,
            tc.tile_pool(name="lnw", bufs=4) as lnw,
            tc.tile_pool(name="expp", bufs=3) as expp,
            tc.tile_pool(name="psA", bufs=2, space="PSUM") as psA,
            tc.tile_pool(name="psB", bufs=2, space="PSUM") as psB,
            tc.tile_pool(name="psC", bufs=2, space="PSUM") as psC,
            tc.tile_pool(name="dram", bufs=1, space="DRAM") as dram,
        ):
            # ---- constants / identity / warm-up --------------------------
            eps_t = consts.tile([128, 1], F32)
            nc.vector.memset(eps_t, 1e-6)
            ones512 = consts.tile([128, 1], BF16)
            nc.vector.memset(ones512, 1.0 / 512.0)
            ident = consts.tile([128, 128], BF16)
            from concourse.masks import make_identity
            make_identity(nc, ident[:])

            # dummy matmul burst: ~3.5us of PE activity trips the HAM clock
            # gate to 2.4 GHz while the first weight/x DMAs are in flight
            warm = psC.tile([128, 128], F32, tag="psC", name="warmup")
            for wi in range(28):
                nc.tensor.matmul(
                    warm[:], lhsT=ident[:], rhs=ident[:],
                    start=(wi == 0), stop=(wi == 27),
                )

            def load_const(shape, src, tag):
                t = consts.tile(shape, BF16, tag=tag)
                nc.gpsimd.dma_start(t[:], src[:])
                return t

            # attention-critical weights first, MLP weights + residual last
            wq_sb = load_const([128, 4, HG * D], wq, "wq_sb")
            wk_sb = load_const([128, 4, HG * D], wk, "wk_sb")
            wv_sb = load_const([128, 4, HG * D], wv, "wv_sb")
            masks_sb = load_const([128, 4, 512], masks, "masks_sb")
            wo1_sb = load_const([128, 2, E], wo1, "wo1_sb")
            wo2_sb = load_const([128, 4, E], wo2, "wo2_sb")
            w1_sb = load_const([128, 4, 4 * E], w1, "w1_sb")
            w2_sb = load_const([128, 16, E], w2, "w2_sb")
            xqT_sb = acts.tile([128, 4, SQ], F32)
            nc.gpsimd.dma_start(xqT_sb[:], xqT[:])

            def layernorm_tile(src_ap, dst_tile):
                """dst (bf16) = (src - mean) * rsqrt(var + eps); stats in fp32."""
                st6 = stats.tile([128, 6], F32, tag="st6")
                nc.vector.bn_stats(st6[:], src_ap)
                mv = stats.tile([128, 2], F32, tag="mv")
                nc.vector.bn_aggr(mv[:], st6[:])
                lnv = stats.tile([128, 1], F32, tag="lnv")
                nc.scalar.activation(lnv[:], mv[:, 1:2], AF.Ln, bias=eps_t[:])
                rsig = stats.tile([128, 1], F32, tag="rsig")
                nc.scalar.activation(rsig[:], lnv[:], AF.Exp, scale=-0.5)
                nc.vector.tensor_scalar(
                    dst_tile[:], src_ap, mv[:, 0:1], rsig[:],
                    op0=ALU.subtract, op1=ALU.mult,
                )

            # ---- LN1 + per-block transpose + QKV -------------------------
            # x arrives through a 2-deep ring of 512-token chunks (prefetch
            # one block ahead) instead of a persistent [128,16,512] tile.
            h1T = acts.tile([128, 4, S], BF16)
            KT = acts.tile([128, 2, S], BF16)
            QT = acts.tile([128, 2, S], BF16)
            V65 = acts.tile([128, NT, HG, D + 1], BF16)
            nc.vector.memset(V65[:, :, :, D:D + 1], 1.0)

            def xf_fetch(st):
                xch = xring.tile([128, 4, E], BF16, tag="xch", name=f"xch{st}")
                nc.sync.dma_start(xch[:], xf[:, 4 * st:4 * st + 4, :])
                return xch

            xch_cur = xf_fetch(0)
            for st in range(4):
                xch = xch_cur
                if st < 3:
                    xch_cur = xf_fetch(st + 1)
                h1ts0 = []
                for lt4 in range(4):
                    t = 4 * st + lt4
                    h1t = lnw.tile([128, E], BF16, tag="lnt", name=f"h1t{t}")
                    layernorm_tile(xch[:, lt4, :], h1t)
                    h1ts0.append(h1t)
                for lt in range(4):
                    for ko in range(4):
                        psT = psC.tile([128, 128], BF16, tag="psC",
                                       name=f"psH{st}_{lt}_{ko}")
                        nc.tensor.transpose(
                            psT[:], h1ts0[lt][:, ko * 128:(ko + 1) * 128], ident[:]
                        )
                        nc.vector.tensor_copy(
                            h1T[:, ko, st * 512 + lt * 128:st * 512 + (lt + 1) * 128],
                            psT[:],
                        )
                sl = slice(st * 512, (st + 1) * 512)
                for mi in range(2):
                    for dst, w_sb in ((KT, wk_sb), (QT, wq_sb)):
                        ps = psC.tile([128, 512], F32, tag="psC")
                        for ko in range(4):
                            nc.tensor.matmul(
                                ps[:],
                                lhsT=w_sb[:, ko, mi * 128:(mi + 1) * 128],
                                rhs=h1T[:, ko, sl],
                                start=(ko == 0), stop=(ko == 3),
                            )
                        nc.vector.tensor_copy(dst[:, mi, sl], ps[:])
                for tt in range(4 * st, 4 * st + 4):
                    ps = psC.tile([128, 512], F32, tag="psC")
                    for ko in range(4):
                        nc.tensor.matmul(
                            ps[:, 0:HG * D],
                            lhsT=h1T[:, ko, tt * 128:(tt + 1) * 128],
                            rhs=wv_sb[:, ko, :],
                            start=(ko == 0), stop=(ko == 3),
                        )
                    nc.vector.tensor_copy(
                        V65[:, tt, :, 0:D],
                        ps[:, 0:HG * D].rearrange("p (h d) -> p h d", h=HG),
                    )

            # ---- attention state + E-major back-half tiles ---------------
            attnT = acts.tile([128, 2, S], BF16)
            # RS layout: row ((qt*2 + g)*E + e), 256 token cols — axis-0 split
            # of each block's [2*E, 256] region hands pair-rank g its tokens
            o1T_dram = dram.tile([QTS * 2 * E, 256], BF16)
            o1rT_dram = dram.tile([QTS * E, 256], BF16)
            o1rT = acts.tile([128, 4, SQ], BF16)
            h2Tb = acts.tile([128, 4, SQ], BF16)
            h2ln = acts.tile([128, 4, SQ], BF16)
            m1T = acts.tile([128, 16, 512], BF16)   # one token-half at a time

            def attention_block(qt, bg):
                """bg: list of closures; one is issued per (a, chunk) slot to
                fill the PE slack under the exp stream."""
                ext = 4 * (qt + 1)           # causal extent in 128-chunks
                for a in range(2):           # local head pairs (2a, 2a+1)
                    avA = psB.tile([D + 1, 512], F32, tag="psB")
                    avB = psB.tile([D + 1, 512], F32, tag="psB")
                    for c in range(ext):
                        sp = psA.tile([128, 1024], F32, tag="psA")
                        nc.tensor.matmul(
                            sp[:, 0:512],
                            lhsT=KT[0:64, a, c * 128:(c + 1) * 128],
                            rhs=QT[0:64, a, qt * 512:(qt + 1) * 512],
                            start=True, stop=True,
                        )
                        nc.tensor.matmul(
                            sp[:, 512:1024],
                            lhsT=KT[64:128, a, c * 128:(c + 1) * 128],
                            rhs=QT[64:128, a, qt * 512:(qt + 1) * 512],
                            start=True, stop=True,
                        )
                        ex = expp.tile([128, 1024], BF16, tag="ex")
                        nc.scalar.activation(ex[:], sp[:], AF.Exp, scale=D ** -0.5)
                        j = c - 4 * qt
                        if j >= 0:           # diagonal chunk: apply causal mask
                            nc.vector.tensor_mul(ex[:, 0:512], ex[:, 0:512], masks_sb[:, j, :])
                            nc.vector.tensor_mul(ex[:, 512:1024], ex[:, 512:1024], masks_sb[:, j, :])
                        nc.tensor.matmul(
                            avA[:], lhsT=V65[:, c, 2 * a, :], rhs=ex[:, 0:512],
                            start=(c == 0), stop=(c == ext - 1),
                        )
                        nc.tensor.matmul(
                            avB[:], lhsT=V65[:, c, 2 * a + 1, :], rhs=ex[:, 512:1024],
                            start=(c == 0), stop=(c == ext - 1),
                        )
                        if bg:
                            bg.pop(0)()
                    for hh, av in ((2 * a, avA), (2 * a + 1, avB)):
                        # copy PSUM out quickly, then build 1/denominator with
                        # the free dim spread across partitions (reciprocal is
                        # ~6.5 cyc per free-elem per lane, so [1,512] is slow)
                        avs = work.tile([D + 1, 512], F32, tag="avs")
                        nc.vector.tensor_copy(avs[:], av[:])
                        d4 = den.tile([8, 64], F32, tag="d4")
                        nc.sync.dma_start(
                            d4[:], avs[D:D + 1, :].rearrange("o (p f) -> o p f", p=8)
                        )
                        r4 = den.tile([8, 64], F32, tag="r4")
                        nc.vector.reciprocal(r4[:], d4[:])
                        rrow = den.tile([1, 512], F32, tag="rrow")
                        nc.sync.dma_start(
                            rrow.rearrange("o (p f) -> o p f", p=8), r4[:]
                        )
                        den_b = work.tile([64, 512], F32, tag="denb")
                        nc.gpsimd.partition_broadcast(den_b[:], rrow[0:1, :], channels=64)
                        if hh % 2 == 0:
                            nc.vector.tensor_tensor(
                                attnT[0:64, a, qt * 512:(qt + 1) * 512],
                                avs[0:D, :], den_b[:], op=ALU.mult,
                            )
                        else:
                            tmp = work.tile([64, 512], BF16, tag="atmp")
                            nc.vector.tensor_tensor(tmp[:], avs[0:D, :], den_b[:], op=ALU.mult)
                            nc.sync.dma_start(attnT[64:128, a, qt * 512:(qt + 1) * 512], tmp[:])

            # ---- E-major wo1 + per-block ReduceScatter -------------------
            def wo1_eo_group(qt, eo):
                """o1T[eo-chunk, block qt] = sum_a wo1_a^T @ attnT_a, then
                cast + store to the token-split RS layout."""
                ps = psC.tile([128, 512], F32, tag="psC")
                for a in range(2):
                    nc.tensor.matmul(
                        ps[:],
                        lhsT=wo1_sb[:, a, eo * 128:(eo + 1) * 128],
                        rhs=attnT[:, a, qt * 512:(qt + 1) * 512],
                        start=(a == 0), stop=(a == 1),
                    )
                o1t = work.tile([128, 512], BF16, tag="wbf")
                nc.vector.tensor_copy(o1t[:], ps[:])
                for g in range(2):
                    base = (qt * 2 + g) * E + eo * 128
                    nc.gpsimd.dma_start(
                        o1T_dram[base:base + 128, :],
                        o1t[:, g * 256:(g + 1) * 256],
                    )

            def rs_block(qt):
                nc.gpsimd.collective_compute(
                    "ReduceScatter", ALU.add,
                    replica_groups=[[0, 1], [2, 3], [4, 5], [6, 7]],
                    ins=[o1T_dram[qt * 2 * E:(qt + 1) * 2 * E, :].opt()],
                    outs=[o1rT_dram[qt * E:(qt + 1) * E, :].opt()],
                )

            def o1rT_load(qt):
                nc.gpsimd.dma_start(
                    o1rT[:, :, qt * 256:(qt + 1) * 256],
                    o1rT_dram[qt * E:(qt + 1) * E, :].rearrange(
                        "(eo p) t -> p eo t", p=128
                    ),
                )

            # ---- E-major wo2 + LN2 (token half h = 512 cols) -------------
            def wo2_half_eo(h, eo, part):
                """half of one eo-chunk of wo2 (2 MMs) — drip-sized."""
                if part == 0:
                    ps = psC.tile([128, 512], F32, tag="psC", name=f"wo2_{h}_{eo}")
                    wo2_half_eo.ps[(h, eo)] = ps
                else:
                    ps = wo2_half_eo.ps.pop((h, eo))
                for ei in (0, 1) if part == 0 else (2, 3):
                    nc.tensor.matmul(
                        ps[:],
                        lhsT=wo2_sb[:, ei, eo * 128:(eo + 1) * 128],
                        rhs=o1rT[:, ei, h * 512:(h + 1) * 512],
                        start=(ei == 0), stop=(ei == 3),
                    )
                if part == 1:
                    nc.vector.tensor_tensor(
                        h2Tb[:, eo, h * 512:(h + 1) * 512],
                        ps[:], xqT_sb[:, eo, h * 512:(h + 1) * 512], op=ALU.add,
                    )
            wo2_half_eo.ps = {}

            def ln2_stats_mm(h, which):
                """colsum matmuls: which=0 -> mean, which=1 -> E[x^2]."""
                ps = psC.tile([1, 512], F32, tag="psC", name=f"ln2s_{h}_{which}")
                if which == 0:
                    for ei in range(4):
                        nc.tensor.matmul(
                            ps[:], lhsT=ones512[:],
                            rhs=h2Tb[:, ei, h * 512:(h + 1) * 512],
                            start=(ei == 0), stop=(ei == 3),
                        )
                    ln2_stats_mm.mu[h] = ps
                else:
                    for ei in range(4):
                        sq = work.tile([128, 512], BF16, tag="sqw")
                        nc.vector.tensor_tensor(
                            sq[:], h2Tb[:, ei, h * 512:(h + 1) * 512],
                            h2Tb[:, ei, h * 512:(h + 1) * 512], op=ALU.mult,
                        )
                        nc.tensor.matmul(
                            ps[:], lhsT=ones512[:], rhs=sq[:],
                            start=(ei == 0), stop=(ei == 3),
                        )
                    ln2_stats_mm.ex2[h] = ps
            ln2_stats_mm.mu = {}
            ln2_stats_mm.ex2 = {}

            def ln2_rowmath(h):
                # sequential [1,512] scratch shares one ring tag (each alloc
                # only reads the previous one, so a 2-deep ring is safe)
                ps_mu = ln2_stats_mm.mu.pop(h)
                ps_ex2 = ln2_stats_mm.ex2.pop(h)
                mu_sb = den.tile([1, 512], BF16, tag="musb", name=f"musb{h}")
                nc.vector.tensor_copy(mu_sb[:], ps_mu[:])
                # two PSUM sources on one DVE op are illegal, hence mu_sb
                musq = den.tile([1, 512], F32, tag="row", name=f"musq{h}")
                nc.vector.tensor_tensor(musq[:], mu_sb[:], mu_sb[:], op=ALU.mult)
                var_s = den.tile([1, 512], F32, tag="row", name=f"vars{h}")
                nc.vector.tensor_tensor(var_s[:], ps_ex2[:], musq[:], op=ALU.subtract)
                lnv = den.tile([1, 512], F32, tag="row", name=f"lnv2{h}")
                nc.scalar.activation(lnv[:], var_s[:], AF.Ln, bias=eps_t[0:1])
                r_f = den.tile([1, 512], F32, tag="row", name=f"rf{h}")
                nc.scalar.activation(r_f[:], lnv[:], AF.Exp, scale=-0.5)
                mur = den.tile([1, 512], BF16, tag="mur")
                nc.vector.tensor_tensor(mur[:], mu_sb[:], r_f[:], op=ALU.mult)
                r_bf = den.tile([1, 512], BF16, tag="rbf")
                nc.vector.tensor_copy(r_bf[:], r_f[:])
                r_b = den.tile([128, 512], BF16, tag="rb", name=f"rb{h}")
                nc.gpsimd.partition_broadcast(r_b[:], r_bf[0:1, :], channels=128)
                mur_b = den.tile([128, 512], BF16, tag="murb", name=f"murb{h}")
                nc.gpsimd.partition_broadcast(mur_b[:], mur[0:1, :], channels=128)
                ln2_rowmath.bcast[h] = (r_b, mur_b)
            ln2_rowmath.bcast = {}

            def ln2_apply_eo(h, eo):
                r_b, mur_b = ln2_rowmath.bcast[h]
                tmp = work.tile([128, 512], BF16, tag="sqw")
                nc.vector.tensor_tensor(
                    tmp[:], h2Tb[:, eo, h * 512:(h + 1) * 512], r_b[:], op=ALU.mult
                )
                nc.vector.tensor_tensor(
                    h2ln[:, eo, h * 512:(h + 1) * 512], tmp[:], mur_b[:],
                    op=ALU.subtract,
                )

            # ---- MLP (E-major, weight-stationary) ------------------------
            def m1_half(h):
                for mi in range(16):
                    ps = psA.tile([128, 512], F32, tag="psA", name=f"m1_{h}_{mi}")
                    for ko in range(4):
                        nc.tensor.matmul(
                            ps[:],
                            lhsT=w1_sb[:, ko, mi * 128:(mi + 1) * 128],
                            rhs=h2ln[:, ko, h * 512:(h + 1) * 512],
                            start=(ko == 0), stop=(ko == 3),
                        )
                    nc.scalar.activation(m1T[:, mi, :], ps[:], AF.Gelu_apprx_tanh)

            def m2_half(h):
                for eo in range(4):
                    ps = psC.tile([128, 512], F32, tag="psC", name=f"m2_{h}_{eo}")
                    for ko in range(16):
                        nc.tensor.matmul(
                            ps[:],
                            lhsT=w2_sb[:, ko, eo * 128:(eo + 1) * 128],
                            rhs=m1T[:, ko, :],
                            start=(ko == 0), stop=(ko == 15),
                        )
                    ob = work.tile([128, 512], F32, tag="avs", name=f"ob{h}_{eo}")
                    nc.vector.tensor_tensor(
                        ob[:], ps[:], xqT_sb[:, eo, h * 512:(h + 1) * 512], op=ALU.add
                    )
                    nc.gpsimd.dma_start(out[:, eo, h * 512:(h + 1) * 512], ob[:])

            # ---- schedule ------------------------------------------------
            # drips are padded with no-op slots so a drip's producer (the
            # previous block's attnT / an RS) is finished before the PE — an
            # in-order engine — reaches the drip's matmuls and would stall.
            nop = lambda: None
            attention_block(0, [])
            attention_block(1, [nop] * 8
                            + [lambda eo=eo: wo1_eo_group(0, eo) for eo in range(4)]
                            + [lambda: rs_block(0)])
            attention_block(2, [nop] * 8
                            + [lambda eo=eo: wo1_eo_group(1, eo) for eo in range(4)]
                            + [lambda: rs_block(1)]
                            + [lambda: o1rT_load(0), lambda: o1rT_load(1)])
            bg3 = [nop] * 4
            bg3 += [lambda eo=eo: wo1_eo_group(2, eo) for eo in range(4)]
            bg3 += [lambda: rs_block(2)]
            for eo in range(4):
                bg3 += [lambda eo=eo: wo2_half_eo(0, eo, 0),
                        lambda eo=eo: wo2_half_eo(0, eo, 1)]
            bg3 += [lambda: ln2_stats_mm(0, 0), lambda: ln2_stats_mm(0, 1)]
            bg3 += [lambda: ln2_rowmath(0)]
            bg3 += [lambda eo=eo: ln2_apply_eo(0, eo) for eo in range(4)]
            attention_block(3, bg3)

            # ---- tail ----------------------------------------------------
            for eo in range(4):
                wo1_eo_group(3, eo)
            rs_block(3)
            o1rT_load(2)
            o1rT_load(3)
            m1_half(0)                      # runs on PE while RS(3) is in flight
            for eo in range(4):
                wo2_half_eo(1, eo, 0)
                wo2_half_eo(1, eo, 1)
            ln2_stats_mm(1, 0)
            ln2_stats_mm(1, 1)
            ln2_rowmath(1)
            for eo in range(4):
                ln2_apply_eo(1, eo)         # DVE — issued before m2(0)'s adds
            m2_half(0)                      # so it overlaps m2(0)'s matmuls
            m1_half(1)
            m2_half(1)

    nc.finalize()
    return nc


@functools.lru_cache(maxsize=1)
def _get_graph():
    return _build_graph()


def _bf16_kpm(a, p=128):
    """[K, M] fp32 -> contiguous [p, K//p, M] bf16 (SBUF (k p) layout)."""
    k, m = a.shape
    return np.ascontiguousarray(
        a.reshape(k // p, p, m).transpose(1, 0, 2)
    ).astype(ml_dtypes.bfloat16)


def _own_rows(rank):
    """Global row indices owned by a core after the per-block reduce-scatters."""
    return np.concatenate(
        [np.arange(512 * qt + 256 * rank, 512 * qt + 256 * rank + 256) for qt in range(QTS)]
    )


def _make_in_maps(x, wq, wk, wv, wo1, wo2, w1, w2, ln1_scale, ln2_scale):
    x = np.asarray(x, dtype=np.float32)
    wq = np.asarray(wq, dtype=np.float32).reshape(E, H * D)
    wk = np.asarray(wk, dtype=np.float32).reshape(E, H * D)
    wv = np.asarray(wv, dtype=np.float32).reshape(E, H * D)
    wo1 = np.asarray(wo1, dtype=np.float32).reshape(H * D, E)
    wo2 = np.asarray(wo2, dtype=np.float32)
    w1 = np.asarray(w1, dtype=np.float32)
    w2 = np.asarray(w2, dtype=np.float32)
    s1 = np.asarray(ln1_scale, dtype=np.float32)[:, None]
    s2 = np.asarray(ln2_scale, dtype=np.float32)[:, None]

    wq_s, wk_s, wv_s = s1 * wq, s1 * wk, s1 * wv
    w1_s = s2 * w1

    # causal mask patterns for diagonal 128-chunks within a 512 q-tile:
    # mask_j[p, f] = 1.0 iff (128j + p) <= f;  stored [p, j, f]
    iota_p = np.arange(128)[:, None]
    iota_f = np.arange(512)[None, :]
    mask_np = np.ascontiguousarray(np.stack(
        [(128 * j + iota_p <= iota_f).astype(np.float32) for j in range(4)]
    ).transpose(1, 0, 2)).astype(ml_dtypes.bfloat16)

    in_maps = []
    for c in range(8):
        b, g = c // 2, c % 2
        hd = slice(g * HG * D, (g + 1) * HG * D)
        rows = _own_rows(c % 2)
        # E-major residual for the owned rows: [128, 4, SQ] with e = eo*128+p
        xqT_arr = np.ascontiguousarray(
            x[b][rows].T.reshape(4, 128, SQ).transpose(1, 0, 2)
        )
        in_maps.append({
            "xf": np.ascontiguousarray(x[b].reshape(NT, 128, E).transpose(1, 0, 2)).astype(ml_dtypes.bfloat16),
            "xqT": xqT_arr,
            "wq": _bf16_kpm(wq_s[:, hd]),
            "wk": _bf16_kpm(wk_s[:, hd]),
            "wv": _bf16_kpm(wv_s[:, hd]),
            "wo1": _bf16_kpm(wo1[hd, :]),
            "wo2": _bf16_kpm(wo2),
            "w1": _bf16_kpm(w1_s),
            "w2": _bf16_kpm(w2),
            "masks": mask_np,
        })
    return in_maps


def run(trace=False, **inputs):
    nc = _get_graph()
    in_maps = _make_in_maps(**inputs)
    res = run_bass_kernel_spmd(nc, in_maps, core_ids=list(range(8)), trace=trace)
    y = np.empty((B, S, E), dtype=np.float32)
    for c in range(8):
        b = c // 2
        # out is E-major [128 p, 4 eo, SQ t]; rows want [t, eo*128+p]
        y[b][_own_rows(c % 2)] = res.results[c]["out"].transpose(2, 1, 0).reshape(SQ, E)
    return y, res


def kernel(**inputs):
    y, _ = run(trace=False, **inputs)
    return y
